# revision 10
# baseline (speedup 1.0000x reference)
"""Trainium2 Bass kernel for nn_ComposerModel (MoE-routing composer).

Strategy: data-parallel over batch across 8 NeuronCores (8 samples/core).
Device program (per core, identical; only inputs differ):
  - fp32 indirect-DMA gather of embedding rows + PE transpose -> hT [D, 256]
  - 4 routed-MLP depths: bf16 matmuls (fp32 PSUM), exact-erf GELU on ACT,
    routing softmax in fp32, per-(sample,prim) scaling folded into u1 before
    the second matmul so PSUM accumulates the weighted combine for free.
    w1 resident in SBUF (bf16), w2 streamed per (depth, prim).
  - LayerNorm via ones-matmul column sums, then vocab head streamed in
    512-wide bf16 chunks, fp32 output.
KL-div scalar is recomputed on host from the per-core routing weights
(tiny: DEP*B*P values) during unsharding.
"""

import numpy as np
import ml_dtypes

B, T, V, D, P, DEP = 64, 32, 32000, 1024, 8, 4
EPS = 1e-5
NCORES = 8
BL = B // NCORES            # samples per core
BT = BL * T                 # tokens per core
DT = D // 128               # 128-partition tiles over D
VW = 512                    # head vocab tile width

_bf16 = ml_dtypes.bfloat16
_CACHE = {}


def _split_multiwaits(nc):
    """This walrus build allows only ONE sync-wait per instruction.  Move all
    but the last wait of any multi-wait instruction onto same-engine nofuse
    nops inserted immediately before it (engine streams run in order, so the
    sequential waits are equivalent)."""
    from concourse import mybir

    n = 0
    for f in nc.m.functions:
        for blk in f.blocks:
            il = blk.instructions
            i = 0
            while i < len(il):
                inst = il[i]
                si = getattr(inst, "sync_info", None)
                if si is not None and len(si.on_wait) > 1:
                    waits = list(si.on_wait)
                    nops = [
                        mybir.InstNoOp(
                            name=f"{inst.name}-wsplit{k}",
                            engine=inst.engine,
                            sync_info=mybir.SyncInfo(on_wait=[w], on_update=[]),
                            bass_nofuse=True,
                        )
                        for k, w in enumerate(waits[:-1])
                    ]
                    inst.sync_info = mybir.SyncInfo(
                        on_wait=[waits[-1]], on_update=list(si.on_update)
                    )
                    il[i:i] = nops
                    i += len(nops)
                    n += 1
                i += 1
    return n


def _build_program(v=V, vw=VW, split=True):
    import concourse.bass as bass
    import concourse.tile as tile
    from concourse import masks, mybir

    f32 = mybir.dt.float32
    bf = mybir.dt.bfloat16
    i32 = mybir.dt.int32
    A = mybir.ActivationFunctionType
    OP = mybir.AluOpType

    nc = bass.Bass()
    xi = nc.dram_tensor("xi", [BT], i32, kind="ExternalInput")
    emb = nc.dram_tensor("emb", [V, D], f32, kind="ExternalInput")
    posr = nc.dram_tensor("posr", [128, DT, T], f32, kind="ExternalInput")
    w1t = nc.dram_tensor("w1t", [128, P * DT * DT * 128], bf, kind="ExternalInput")
    w2t = nc.dram_tensor("w2t", [P, 128, DT * DT * 128], bf, kind="ExternalInput")
    selt = nc.dram_tensor("selt", [128, DEP, DT, P], bf, kind="ExternalInput")
    b1t = nc.dram_tensor("b1t", [128, P, DT], f32, kind="ExternalInput")
    b2t = nc.dram_tensor("b2t", [P, DT, 128], bf, kind="ExternalInput")
    lngb = nc.dram_tensor("lngb", [128, 2, DT], f32, kind="ExternalInput")
    gum = nc.dram_tensor("gum", [BL, DEP, P], f32, kind="ExternalInput")
    onehot_d = nc.dram_tensor("onehot", [BL, BT], bf, kind="ExternalInput")
    headt = nc.dram_tensor("headt", [128, DT, v], bf, kind="ExternalInput")
    out = nc.dram_tensor("out", [BT, v], f32, kind="ExternalOutput")
    wout = nc.dram_tensor("wout", [BL, DEP, P], f32, kind="ExternalOutput")

    def bcast(ap2d, count, pos):
        """Insert a step-0 broadcast dim of size `count` at free position."""
        ap = list(ap2d.ap)
        ap.insert(pos, [0, count])
        return bass.AP(tensor=ap2d.tensor, offset=ap2d.offset, ap=ap)

    with tile.TileContext(nc) as tc:
        with tc.tile_pool(name="const", bufs=1) as cp, \
             tc.tile_pool(name="res", bufs=1) as rp:
            ident_f = cp.tile([128, 128], f32)
            masks.make_identity(nc, ident_f[:])
            ident_b = cp.tile([128, 128], bf)
            masks.make_identity(nc, ident_b[:])
            ones_col_b = cp.tile([128, 1], bf)       # for column sums
            nc.vector.memset(ones_col_b[:], 1.0)
            ones_row_f = cp.tile([1, 128], f32)      # for partition bcast (f32)
            nc.vector.memset(ones_row_f[:], 1.0)
            ones8 = cp.tile([BL, 128], bf)           # K=BL bcast helper
            nc.vector.memset(ones8[:], 1.0)
            onehot = cp.tile([BL, BT], bf)           # onehot[b, b*T:(b+1)*T] = 1
            nc.sync.dma_start(out=onehot[:], in_=onehot_d[:])

            # resident tensors
            w1s = rp.tile([128, P, DT, DT, 128], bf)
            for p in range(P):
                nc.sync.dma_start(
                    out=w1s[:, p, :, :, :],
                    in_=w1t[:, p * DT * DT * 128:(p + 1) * DT * DT * 128]
                        .rearrange("a (k d c) -> a k d c", k=DT, d=DT),
                )
            sel_s = rp.tile([128, DEP, DT, P], bf)
            nc.sync.dma_start(out=sel_s[:], in_=selt[:])
            b1s = rp.tile([128, P, DT], f32)
            nc.sync.dma_start(out=b1s[:], in_=b1t[:])
            b2s = rp.tile([P, DT, 128], bf)
            nc.sync.dma_start(out=b2s[:], in_=b2t[:])
            lngbs = rp.tile([128, 2, DT], f32)
            nc.sync.dma_start(out=lngbs[:], in_=lngb[:])
            gums = rp.tile([BL, DEP, P], f32)
            nc.sync.dma_start(out=gums[:], in_=gum[:])
            poss = rp.tile([128, DT, T], f32)
            nc.sync.dma_start(out=poss[:], in_=posr[:])
            hT = rp.tile([128, DT, BT], f32)
            wouts = rp.tile([BL, DEP, P], f32)

            # ---- embedding gather + transpose + pos add ----
            with tc.tile_pool(name="init", bufs=2) as ip, \
                 tc.tile_pool(name="psI", bufs=2, space="PSUM") as psI:
                for tt in range(BT // 128):
                    idx = ip.tile([128, 1], i32, tag="idx")
                    nc.sync.dma_start(out=idx[:], in_=xi[tt * 128:(tt + 1) * 128, None])
                    rows = ip.tile([128, D], f32, tag="rows")
                    nc.gpsimd.indirect_dma_start(
                        out=rows[:],
                        out_offset=None,
                        in_=emb[:],
                        in_offset=bass.IndirectOffsetOnAxis(ap=idx[:, :1], axis=0),
                    )
                    spt = 128 // T          # samples per token-tile
                    for k in range(DT):
                        ptp = psI.tile([128, 128], f32, tag="i")
                        nc.tensor.transpose(ptp[:], rows[:, k * 128:(k + 1) * 128], ident_f[:])
                        dst = hT[:, k, tt * 128:(tt + 1) * 128]
                        dst3 = dst.rearrange("a (b t) -> a b t", t=T)
                        nc.vector.tensor_tensor(
                            out=dst3,
                            in0=ptp[:].rearrange("a (b t) -> a b t", t=T),
                            in1=bcast(poss[:, k, :], spt, 1),
                            op=OP.add,
                        )

            # ---- depth loop ----
            with tc.tile_pool(name="wk", bufs=1) as wk, \
                 tc.tile_pool(name="u1p", bufs=2) as u1p, \
                 tc.tile_pool(name="dwp", bufs=2) as dwp, \
                 tc.tile_pool(name="psA", bufs=3, space="PSUM") as psA, \
                 tc.tile_pool(name="psB", bufs=3, space="PSUM") as psB, \
                 tc.tile_pool(name="psS", bufs=2, space="PSUM") as psS:

                for dep in range(DEP):
                    hbf = wk.tile([128, DT, BT], bf, tag="hbf")
                    nc.vector.tensor_copy(out=hbf[:], in_=hT[:])
                    ctx = wk.tile([128, DT, BL], f32, tag="ctx")
                    for k in range(DT):
                        nc.vector.tensor_reduce(
                            out=ctx[:, k, :],
                            in_=hT[:, k, :].rearrange("a (b t) -> a b t", t=T),
                            axis=mybir.AxisListType.X,
                            op=OP.add,
                        )
                    ctxb = wk.tile([128, DT, BL], bf, tag="ctxb")
                    nc.scalar.mul(out=ctxb[:], in_=ctx[:], mul=1.0 / T)

                    psel = psS.tile([BL, P], f32, tag="s")
                    for k in range(DT):
                        nc.tensor.matmul(psel[:], ctxb[:, k, :], sel_s[:, dep, k, :],
                                         start=(k == 0), stop=(k == DT - 1))
                    ls = wk.tile([BL, P], f32, tag="ls")
                    nc.vector.tensor_add(out=ls[:], in0=psel[:], in1=gums[:, dep, :])
                    ngm = wk.tile([BL, 1], f32, tag="ngm")
                    nc.vector.tensor_reduce(out=ngm[:], in_=ls[:],
                                            axis=mybir.AxisListType.X,
                                            op=OP.max, negate=True)
                    ex = wk.tile([BL, P], f32, tag="ex")
                    den = wk.tile([BL, 1], f32, tag="den")
                    nc.scalar.activation(out=ex[:], in_=ls[:], func=A.Exp,
                                         bias=ngm[:], scale=1.0, accum_out=den[:])
                    rec = wk.tile([BL, 1], f32, tag="rec")
                    nc.vector.reciprocal(out=rec[:], in_=den[:])
                    wgt = wk.tile([BL, P], f32, tag="wgt")
                    nc.vector.tensor_scalar_mul(out=wgt[:], in0=ex[:], scalar1=rec[:])
                    nc.vector.tensor_copy(out=wouts[:, dep, :], in_=wgt[:])
                    wgb = wk.tile([BL, P], bf, tag="wgb")
                    nc.vector.tensor_copy(out=wgb[:], in_=wgt[:])
                    pwt = psS.tile([BL, P], bf, tag="s")
                    nc.tensor.transpose(pwt[:], wgb[:], ident_b[:BL, :BL])
                    wtb = wk.tile([P, BL], bf, tag="wtb")
                    nc.vector.tensor_copy(out=wtb[:], in_=pwt[:])

                    scl = wk.tile([128, P, BT], bf, tag="scl")
                    b2w = wk.tile([128, DT, BL], f32, tag="b2w")

                    def layer1_mm_gelu(p, u1s):
                        for do in range(DT):
                            ps = psA.tile([128, BT], f32, tag="a")
                            for k in range(DT):
                                nc.tensor.matmul(ps[:], w1s[:, p, k, do, :], hbf[:, k, :],
                                                 start=(k == 0), stop=(k == DT - 1))
                            nc.scalar.activation(out=u1s[:, do, :], in_=ps[:], func=A.Gelu,
                                                 bias=b1s[:, p, do:do + 1], scale=1.0)

                    def layer1_scale(p, u1s):
                        for do in range(DT):
                            nc.vector.tensor_mul(out=u1s[:, do, :], in0=u1s[:, do, :],
                                                 in1=scl[:, p, :])

                    def layer2(p, u1s, w2slot):
                        for do in range(DT):
                            ps = psB.tile([128, BT], f32, tag="b")
                            for k in range(DT):
                                nc.tensor.matmul(ps[:], w2slot[:, k, do, :], u1s[:, k, :],
                                                 start=(k == 0), stop=(k == DT - 1))
                            nc.vector.tensor_add(out=hT[:, do, :], in0=hT[:, do, :], in1=ps[:])

                    # p = 0: matmuls+gelu first so PE stays busy while the
                    # softmax chain finishes on ACT/DVE; then build the scale
                    # tiles (PE) and apply.
                    u1s0 = u1p.tile([128, DT, BT], bf, tag="u1s")
                    w2s0 = dwp.tile([128, DT, DT, 128], bf, tag="w2")
                    nc.sync.dma_start(
                        out=w2s0[:],
                        in_=w2t[0].rearrange("a (k d c) -> a k d c", k=DT, d=DT))
                    layer1_mm_gelu(0, u1s0)
                    for p in range(P):
                        rhp = wk.tile([BL, BT], bf, tag="rhp")
                        nc.vector.tensor_scalar_mul(out=rhp[:], in0=onehot[:],
                                                    scalar1=wgt[:, p:p + 1])
                        psc = psS.tile([128, BT], f32, tag="s")
                        nc.tensor.matmul(psc[:], ones8[:], rhp[:],
                                         start=True, stop=True)
                        nc.vector.tensor_copy(out=scl[:, p, :], in_=psc[:])
                    for k in range(DT):
                        pb2 = psS.tile([128, BL], f32, tag="s")
                        nc.tensor.matmul(pb2[:], b2s[:, k, :], wtb[:], start=True, stop=True)
                        nc.vector.tensor_copy(out=b2w[:, k, :], in_=pb2[:])
                    layer1_scale(0, u1s0)

                    prev = (u1s0, w2s0)
                    for p in range(1, P + 1):
                        if p <= P - 1:
                            u1s = u1p.tile([128, DT, BT], bf, tag="u1s")
                            w2s = dwp.tile([128, DT, DT, 128], bf, tag="w2")
                            nc.sync.dma_start(
                                out=w2s[:],
                                in_=w2t[p].rearrange("a (k d c) -> a k d c", k=DT, d=DT))
                            layer1_mm_gelu(p, u1s)
                            layer1_scale(p, u1s)
                        layer2(p - 1, *prev)
                        if p <= P - 1:
                            prev = (u1s, w2s)

                    # bias2 combine term (b2 is summed with routing weights)
                    for k in range(DT):
                        nc.vector.tensor_tensor(
                            out=hT[:, k, :].rearrange("a (b t) -> a b t", t=T),
                            in0=hT[:, k, :].rearrange("a (b t) -> a b t", t=T),
                            in1=bcast(b2w[:, k, :], T, 2),
                            op=OP.add,
                        )

            # ---- layernorm + vocab head (depth-phase pools closed) ----
            with tc.tile_pool(name="ln", bufs=1) as ln, \
                 tc.tile_pool(name="hd", bufs=3) as hd, \
                 tc.tile_pool(name="ob", bufs=4) as obp, \
                 tc.tile_pool(name="psL", bufs=2, space="PSUM") as psL, \
                 tc.tile_pool(name="psH", bufs=4, space="PSUM") as psH:
                hnb = ln.tile([128, DT, BT], bf, tag="hnb")
                hsq = ln.tile([128, DT, BT], bf, tag="hsq")
                hbf2 = ln.tile([128, DT, BT], bf, tag="hbf2")
                nc.vector.tensor_copy(out=hbf2[:], in_=hT[:])
                nc.vector.tensor_mul(out=hsq[:], in0=hT[:], in1=hT[:])
                pmu = psL.tile([1, BT], f32, tag="s")
                for k in range(DT):
                    nc.tensor.matmul(pmu[:], ones_col_b[:], hbf2[:, k, :],
                                     start=(k == 0), stop=(k == DT - 1))
                pm2 = psL.tile([1, BT], f32, tag="s")
                for k in range(DT):
                    nc.tensor.matmul(pm2[:], ones_col_b[:], hsq[:, k, :],
                                     start=(k == 0), stop=(k == DT - 1))
                mean = ln.tile([1, BT], f32, tag="mean")
                nc.scalar.mul(out=mean[:], in_=pmu[:], mul=1.0 / D)
                e2 = ln.tile([1, BT], f32, tag="e2")
                nc.scalar.mul(out=e2[:], in_=pm2[:], mul=1.0 / D)
                msq = ln.tile([1, BT], f32, tag="msq")
                nc.vector.tensor_mul(out=msq[:], in0=mean[:], in1=mean[:])
                vtmp = ln.tile([1, BT], f32, tag="vtmp")
                nc.vector.scalar_tensor_tensor(out=vtmp[:], in0=e2[:], scalar=EPS,
                                               in1=msq[:], op0=OP.add, op1=OP.subtract)
                sd = ln.tile([1, BT], f32, tag="sd")
                nc.scalar.sqrt(out=sd[:], in_=vtmp[:])
                rstd = ln.tile([1, BT], f32, tag="rstd")
                nc.vector.reciprocal(out=rstd[:], in_=sd[:])
                pmb = psL.tile([128, BT], f32, tag="s")
                nc.tensor.matmul(pmb[:], ones_row_f[:], mean[:], start=True, stop=True)
                mbc = ln.tile([128, BT], f32, tag="mbc")
                nc.vector.tensor_copy(out=mbc[:], in_=pmb[:])
                prb = psL.tile([128, BT], f32, tag="s")
                nc.tensor.matmul(prb[:], ones_row_f[:], rstd[:], start=True, stop=True)
                rbc = ln.tile([128, BT], f32, tag="rbc")
                nc.vector.tensor_copy(out=rbc[:], in_=prb[:])
                tnorm = ln.tile([128, BT], f32, tag="tnorm")
                for k in range(DT):
                    nc.vector.tensor_tensor(out=tnorm[:], in0=hT[:, k, :], in1=mbc[:],
                                            op=OP.subtract)
                    nc.vector.tensor_mul(out=tnorm[:], in0=tnorm[:], in1=rbc[:])
                    nc.vector.tensor_scalar(
                        out=hnb[:, k, :], in0=tnorm[:],
                        scalar1=lngbs[:, 0, k:k + 1], scalar2=lngbs[:, 1, k:k + 1],
                        op0=OP.mult, op1=OP.add)

                nc.sync.dma_start(out=wout[:], in_=wouts[:])
                nv = (v + vw - 1) // vw
                for vt in range(nv):
                    v0 = vt * vw
                    w_ = min(vw, v - v0)
                    chunk = hd.tile([128, DT, vw], bf, tag="chunk")
                    nc.sync.dma_start(out=chunk[:, :, :w_], in_=headt[:, :, v0:v0 + w_])
                    for tt in range(BT // 128):
                        ps = psH.tile([128, vw], f32, tag="h")
                        for k in range(DT):
                            nc.tensor.matmul(ps[:, :w_],
                                             hnb[:, k, tt * 128:(tt + 1) * 128],
                                             chunk[:, k, :w_],
                                             start=(k == 0), stop=(k == DT - 1))
                        ob = obp.tile([128, vw], f32, tag="o")
                        nc.vector.tensor_copy(out=ob[:, :w_], in_=ps[:, :w_])
                        nc.sync.dma_start(out=out[tt * 128:(tt + 1) * 128, v0:v0 + w_],
                                          in_=ob[:, :w_])

    if split:
        _split_multiwaits(nc)
    return nc


def _prep_shared(emb, pos, w1, b1, w2, b2, sel_w, sel_b, ln_g, ln_b, head_w, v=V):
    emb = np.asarray(emb, np.float32)
    pos = np.asarray(pos, np.float32)
    posr = np.ascontiguousarray(pos.T.reshape(DT, 128, T).transpose(1, 0, 2))
    w1 = np.asarray(w1, np.float32)
    w1t = np.ascontiguousarray(
        w1.reshape(P, DT, 128, DT, 128).transpose(2, 0, 1, 3, 4)
    ).astype(_bf16).reshape(128, -1)
    w2 = np.asarray(w2, np.float32)
    w2t = np.ascontiguousarray(
        w2.reshape(P, DT, 128, DT, 128).transpose(0, 2, 1, 3, 4)
    ).astype(_bf16).reshape(P, 128, -1)
    sel_w = np.asarray(sel_w, np.float32)
    selt = np.ascontiguousarray(
        sel_w.transpose(2, 0, 1).reshape(DT, 128, DEP, P).transpose(1, 2, 0, 3)
    ).astype(_bf16)
    b1t = np.ascontiguousarray(np.asarray(b1, np.float32).reshape(P, DT, 128)
                               .transpose(2, 0, 1))
    b2t = np.asarray(b2, np.float32).reshape(P, DT, 128).astype(_bf16)
    lngb = np.stack([np.asarray(ln_g, np.float32).reshape(DT, 128).T,
                     np.asarray(ln_b, np.float32).reshape(DT, 128).T], axis=1)
    lngb = np.ascontiguousarray(lngb)          # [128, 2, DT]
    head_w = np.asarray(head_w, np.float32)
    headt = np.ascontiguousarray(
        head_w.T.reshape(DT, 128, v).transpose(1, 0, 2)
    ).astype(_bf16)
    onehot = np.zeros((BL, BT), np.float32)
    for b_ in range(BL):
        onehot[b_, b_ * T:(b_ + 1) * T] = 1.0
    return dict(emb=emb, posr=posr, w1t=w1t, w2t=w2t, selt=selt,
                b1t=b1t, b2t=b2t, lngb=lngb, headt=headt,
                onehot=onehot.astype(_bf16))


def kernel(x, emb, pos, w1, b1, w2, b2, sel_w, sel_b, ln_g, ln_b, head_w, gumbel):
    from concourse.bass_utils import run_bass_kernel_spmd

    if "nc" not in _CACHE:
        _CACHE["nc"] = _build_program()
    nc = _CACHE["nc"]

    shared = _prep_shared(emb, pos, w1, b1, w2, b2, sel_w, sel_b, ln_g, ln_b, head_w)
    x = np.asarray(x)
    g2 = np.asarray(gumbel, np.float32) + np.asarray(sel_b, np.float32)[:, None, :]

    in_maps = []
    for c in range(NCORES):
        m = dict(shared)
        m["xi"] = np.ascontiguousarray(
            x[c * BL:(c + 1) * BL, :].reshape(-1).astype(np.int32))
        m["gum"] = np.ascontiguousarray(
            g2[:, c * BL:(c + 1) * BL, :].transpose(1, 0, 2))
        in_maps.append(m)

    res = run_bass_kernel_spmd(nc, in_maps, list(range(NCORES)))

    logits = np.concatenate(
        [res.results[c]["out"].reshape(BL, T, V) for c in range(NCORES)], axis=0)
    wall = np.concatenate(
        [res.results[c]["wout"].transpose(1, 0, 2) for c in range(NCORES)], axis=1)
    avg = wall.mean(axis=1)                     # [DEP, P] fp32
    unif = np.float32(1.0 / P)
    div = np.float32(0.0)
    for dp in range(DEP):
        div = div + np.sum(unif * (np.log(unif) - np.log(avg[dp])), dtype=np.float32)
    return logits, np.float32(div)
